# revision 42
# baseline (speedup 1.0000x reference)
"""CosRec-style pairwise-MLP recommender kernel for 8 Trainium2 NeuronCores.

Reference computation (per batch element b, L=32, D=64, FC=100):
    embs   = item_emb[seq_var]                      [B, L, D]
    A      = embs @ Wa^T + b1 (Wa = W1[:, :D])      [B, L, FC]
    Bm     = embs @ Wb^T  (Wb = W1[:, D:])          [B, L, FC]
    h1     = relu(A[:,None,:,:] + Bm[:,:,None,:])   [B, L, L, FC]
    h2     = relu(h1 @ Wf2^T + bf2)                 [B, L, L, FC]
    x      = h2.sum((1, 2))                         [B, FC]
    out[b,t] = b2[item_var[b,t]] + W2[item_var[b,t]] . cat(x[b], user_emb[user_var[b]])

Strategy: data-parallel over batch (64 examples/core).  Hard-won placement
rules for this runtime/HW:
  - GpSimd must never stream bulk elementwise data (shares an SBUF port
    with the DVE); it only issues the indirect-DMA gathers.  Multi-column
    offset APs are broken in the SWDGE path, so gathers stay one offset
    column per instruction, issued before anything else on that queue
    (the identity matrix comes in as an input so make_identity's iota
    does not block the gathers).
  - DVE accum_out / tensor_tensor_reduce are broken; only ScalarE
    activation accum_out works => relu2(+bf2)+pair-sum lives on ACT.
  - fp16 Wf2 fails the 2e-2 gate (weight rounding error is systematic
    across the 1024-term positive pair-sum); f32r moving operands at
    >=256 cols already run 1 col/cycle, so everything stays f32r.
  - The outer-sum broadcast ADD can never exceed DVE 1x mode (the Bm
    operand has innermost stride 0). Both L1 evictions run on DVE (ACT
    is the relu2 bottleneck); pa gets +b1 fused via a 2D
    scalar_tensor_tensor.
Main loop per ADD group (1-4 examples, small ramp-in/drain-out groups):
one DVE TT ADD -> pre [100, gs*1024] f32r, relu1 in place (DVE 2x
tensor_scalar or ACT, balance knob), then per example 2 f32r matmuls
into PSUM and one ACT relu+bias+accum -> x column.  Chunks are
software-pipelined: chunk c+1's transposes/L1 are emitted between chunk
c's ADD groups so DVE never waits on L1.  Knob values are phase-critical
(+-10us swings): r1_act_num=5, h2_bufs=3/stage_bufs=2, pre_bufs=6.
"""

import os
import sys

import numpy as np

sys.path.insert(0, "/opt/trn_rl_repo")

import concourse.bass as bass
import concourse.tile as tile
from concourse import bacc, mybir
from concourse.bass_utils import run_bass_kernel_spmd
from contextlib import ExitStack

N_CORES = 8
B_FULL = 512
BPC = B_FULL // N_CORES  # 64 examples per core
L = 32
D = 64
FC = 100
T = 3
NROW = BPC * L           # 2048 gathered rows per core
NTILE = NROW // 128      # 16 gather tiles
F32 = mybir.dt.float32
F32R = mybir.dt.float32r
BF16 = mybir.dt.bfloat16
F16 = mybir.dt.float16
I32 = mybir.dt.int32

# ---- tunables -------------------------------------------------------------
CFG = dict(
    r1_act_num=5,      # of 16 4-example groups, how many run relu1 on ACT
    r1_tail_dve=2,     # last N groups always use DVE relu1 (ACT drains relu2)
    bstep=4,           # examples per ADD/relu1 group
    h2_bufs=3,         # PSUM slots of [100, 1024] (2 banks each)
    stage_bufs=2,      # shared PSUM ring for transposes + L1 out (1 bank each)
    pre_bufs=6,
)

_PROG_CACHE = {}


def _build_program(cfg):
    nc = bacc.Bacc()

    seq_idx = nc.dram_tensor("seq_idx", [128, NTILE], I32, kind="ExternalInput")
    user_idx = nc.dram_tensor("user_idx", [BPC, 1], I32, kind="ExternalInput")
    item_idx = nc.dram_tensor("item_idx", [BPC, T], I32, kind="ExternalInput")
    item_emb = nc.dram_tensor("item_emb", [100000, D], F32, kind="ExternalInput")
    user_emb = nc.dram_tensor("user_emb", [100000, D], F32, kind="ExternalInput")
    W2 = nc.dram_tensor("W2", [100000, FC + D], F32, kind="ExternalInput")
    b2 = nc.dram_tensor("b2", [100000, 1], F32, kind="ExternalInput")
    W1 = nc.dram_tensor("W1", [FC, 2 * D], F32, kind="ExternalInput")
    b1 = nc.dram_tensor("b1", [FC, 1], F32, kind="ExternalInput")
    Wf2 = nc.dram_tensor("Wf2", [FC, FC], F32, kind="ExternalInput")
    bf2 = nc.dram_tensor("bf2", [FC, 1], F32, kind="ExternalInput")
    ident_d = nc.dram_tensor("ident128", [128, 128], F32, kind="ExternalInput")
    out_d = nc.dram_tensor("out", [BPC, T], F32, kind="ExternalOutput")

    Relu = mybir.ActivationFunctionType.Relu
    Ident = mybir.ActivationFunctionType.Identity
    Add = mybir.AluOpType.add
    Mult = mybir.AluOpType.mult

    BS = cfg["bstep"]               # examples per ADD group (steady state)
    # chunk layout (examples per L1 chunk): small first chunks so the first
    # ADD starts as soon as the first gathered tile lands
    CHUNKS = [4, 4] + [8] * 7
    assert sum(CHUNKS) == BPC
    # group sizes per chunk: tiny ramp-in (ACT's first relu2 comes ~6us
    # earlier) and a fine drain-out (shorter serial tail after the last ADD)
    GSIZES = [[1, 1, 2], [2, 2]] + [[4] * (c // 4) for c in CHUNKS[2:-1]] + [
        [2, 2, 2, 1, 1]
    ]
    GROUPS = []  # (chunk, b0, size)
    b0 = 0
    for c, sizes in enumerate(GSIZES):
        assert sum(sizes) == CHUNKS[c]
        for s in sizes:
            GROUPS.append((c, b0, s))
            b0 += s
    NGRP = len(GROUPS)
    # relu1 engine per group: k ACT groups spread over all but the last
    # r1_tail_dve groups (ACT must drain relu2 at the end)
    k = cfg["r1_act_num"]
    nfree = NGRP - cfg["r1_tail_dve"]
    r1_act = [
        g < nfree and ((g + 1) * k) // nfree > (g * k) // nfree for g in range(NGRP)
    ]

    with ExitStack() as ctx:
        tc = ctx.enter_context(tile.TileContext(nc))
        const = ctx.enter_context(tc.tile_pool(name="const", bufs=1))
        prep = ctx.enter_context(tc.tile_pool(name="pre", bufs=cfg["pre_bufs"]))
        scrp = ctx.enter_context(tc.tile_pool(name="scr", bufs=2))
        stage = ctx.enter_context(
            tc.tile_pool(name="stage", bufs=cfg["stage_bufs"], space="PSUM")
        )
        ps2 = ctx.enter_context(
            tc.tile_pool(name="ps2", bufs=cfg["h2_bufs"], space="PSUM")
        )

        # ---------------- gathers first: longest GpSimd-serial chain --------
        idx_sb = const.tile([128, NTILE], I32)
        nc.sync.dma_start(out=idx_sb[:], in_=seq_idx[:, :])
        uidx_sb = const.tile([BPC, 1], I32)
        nc.sync.dma_start(out=uidx_sb[:], in_=user_idx[:, :])
        iidx_sb = const.tile([BPC, T], I32)
        nc.sync.dma_start(out=iidx_sb[:], in_=item_idx[:, :])

        # gathers cast fp32->fp16 in the DMA (SWDGE): halves the SBUF write
        # traffic that contends with DVE's 2-port modes (embedding values in
        # fp16 cost ~1e-3 rel err; the gate is 2e-2)
        g_all = const.tile([128, NTILE * D], F16)
        for t in range(NTILE):
            nc.gpsimd.indirect_dma_start(
                out=g_all[:, t * D : (t + 1) * D],
                out_offset=None,
                in_=item_emb[:, :],
                in_offset=bass.IndirectOffsetOnAxis(ap=idx_sb[:, t : t + 1], axis=0),
            )
        # final-stage gathers queue behind; they overlap the main loop
        ug = const.tile([BPC, D], F32)
        nc.gpsimd.indirect_dma_start(
            out=ug[:],
            out_offset=None,
            in_=user_emb[:, :],
            in_offset=bass.IndirectOffsetOnAxis(ap=uidx_sb[:, 0:1], axis=0),
        )
        w2g = const.tile([BPC, T * (FC + D)], F32)
        for t in range(T):
            nc.gpsimd.indirect_dma_start(
                out=w2g[:, t * (FC + D) : (t + 1) * (FC + D)],
                out_offset=None,
                in_=W2[:, :],
                in_offset=bass.IndirectOffsetOnAxis(ap=iidx_sb[:, t : t + 1], axis=0),
            )
        b2g = const.tile([BPC, T], F32)
        for t in range(T):
            nc.gpsimd.indirect_dma_start(
                out=b2g[:, t : t + 1],
                out_offset=None,
                in_=b2[:, :],
                in_offset=bass.IndirectOffsetOnAxis(ap=iidx_sb[:, t : t + 1], axis=0),
            )

        # ---------------- constants & weights ----------------
        ident = const.tile([128, 128], F32)
        nc.sync.dma_start(out=ident[:], in_=ident_d[:, :])
        ident16 = const.tile([128, 128], F16)
        nc.vector.tensor_copy(ident16[:], ident[:])
        w1_sb = const.tile([FC, 2 * D], F32)
        nc.sync.dma_start(out=w1_sb[:], in_=W1[:, :])
        wf2_sb = const.tile([FC, FC], F32)
        nc.sync.dma_start(out=wf2_sb[:], in_=Wf2[:, :])
        b1_sb = const.tile([FC, 1], F32)
        nc.sync.dma_start(out=b1_sb[:], in_=b1[:, :])
        bf2_sb = const.tile([FC, 1], F32)
        nc.sync.dma_start(out=bf2_sb[:], in_=bf2[:, :])

        # WaT/WbT: [64, 100] = (W1[:, :D]).T and (W1[:, D:]).T
        waT = const.tile([D, FC], F32R)
        wbT = const.tile([D, FC], F32R)
        for half, dst in ((0, waT), (1, wbT)):
            w1h_ps = stage.tile([D, FC], F32, tag="stage")
            nc.tensor.transpose(
                w1h_ps[:], w1_sb[:, half * D : (half + 1) * D], ident[:FC, :FC]
            )
            nc.vector.tensor_copy(dst[:], w1h_ps[:])

        # Wf2T: [100, 100] = Wf2.T
        wf2t_ps = stage.tile([FC, FC], F32, tag="stage")
        nc.tensor.transpose(wf2t_ps[:], wf2_sb[:], ident[:FC, :FC])
        wf2t = const.tile([FC, FC], F32R)
        nc.vector.tensor_copy(wf2t[:], wf2t_ps[:])

        embsT = const.tile([D, NROW], F32R)
        A_sb = const.tile([FC, NROW], F32)   # A' = embs@Wa^T + b1 (bias folded)
        Bm_sb = const.tile([FC, NROW], F32)  # Bm = embs@Wb^T
        x = const.tile([FC, BPC], F32)       # x[:, b] = sum_{a,c} h2[b, a, c, :]
        zeros = const.tile([FC, 1], F32)
        nc.vector.memset(zeros[:], 0.0)

        chunk_b0 = [sum(CHUNKS[:i]) for i in range(len(CHUNKS))]  # first example

        def prep_chunk(chunk):
            """Transpose + layer 1 + DVE evictions for one chunk."""
            cb = CHUNKS[chunk]
            cw = cb * L
            r0 = chunk_b0[chunk] * L          # first gathered row
            tp = stage.tile([D, 256], F16, tag="stage", name=f"tp{chunk}")
            for i in range(cw // 128):
                t0 = r0 // 128 + i
                nc.tensor.transpose(
                    tp[:, i * 128 : (i + 1) * 128],
                    g_all[:, t0 * D : (t0 + 1) * D],
                    ident16[:, :],
                )
            sl = slice(r0, r0 + cw)
            nc.vector.tensor_copy(embsT[:, sl], tp[:, 0:cw])
            l1 = stage.tile([FC, 512], F32, tag="stage", name=f"l1_{chunk}")
            nc.tensor.matmul(
                l1[:, 0:cw], lhsT=waT[:], rhs=embsT[:, sl], start=True, stop=True
            )
            nc.tensor.matmul(
                l1[:, cw : 2 * cw], lhsT=wbT[:], rhs=embsT[:, sl], start=True, stop=True
            )
            nc.vector.scalar_tensor_tensor(
                out=A_sb[:, sl],
                in0=l1[:, 0:cw],
                scalar=b1_sb[:, 0:1],
                in1=zeros[:, 0:1].to_broadcast([FC, cw]),
                op0=Add,
                op1=Add,
            )
            nc.vector.tensor_copy(Bm_sb[:, sl], l1[:, cw : 2 * cw])

        prep_chunk(0)
        prepped = 0
        for grp, (chunk, b, gs) in enumerate(GROUPS):
            pre = prep.tile([FC, gs * L * L], F32R, tag="pre")
            in0 = (
                A_sb[:, b * L : (b + gs) * L]
                .rearrange("p (j c) -> p j c", j=gs)
                .unsqueeze(2)
                .to_broadcast([FC, gs, L, L])
            )
            in1 = (
                Bm_sb[:, b * L : (b + gs) * L]
                .rearrange("p (j a) -> p j a", j=gs)
                .unsqueeze(3)
                .to_broadcast([FC, gs, L, L])
            )
            nc.vector.tensor_tensor(
                out=pre[:].rearrange("p (j a c) -> p j a c", j=gs, a=L),
                in0=in0,
                in1=in1,
                op=Add,
            )
            # relu1 in place, split fine-grained so layer-2 matmuls of
            # example j start right after relu1(j), not the whole group
            if r1_act[grp]:
                nh = max(1, gs // 2)
                half = gs * L * L // nh
                for h in range(nh):
                    nc.scalar.activation(
                        pre[:, h * half : (h + 1) * half],
                        pre[:, h * half : (h + 1) * half],
                        Relu,
                    )
            else:
                nj = 2 if gs == 4 else 1
                step = gs * L * L // nj
                for j in range(nj):
                    nc.vector.tensor_scalar_max(
                        pre[:, j * step : (j + 1) * step],
                        pre[:, j * step : (j + 1) * step],
                        0.0,
                    )
            for j in range(gs):
                bb = b + j
                h2p = ps2.tile([FC, L * L], F32, tag="ps2")
                for h in range(2):
                    nc.tensor.matmul(
                        h2p[:, h * 512 : (h + 1) * 512],
                        lhsT=wf2t[:],
                        rhs=pre[:, j * L * L + h * 512 : j * L * L + (h + 1) * 512],
                        start=True,
                        stop=True,
                    )
                # relu2(+bf2) with fused pair-sum accumulation -> x[:, bb]
                h2s = scrp.tile([FC, L * L], F16, tag="h2s")
                nc.scalar.activation(
                    h2s[:], h2p[:], Relu,
                    bias=bf2_sb[:, 0:1],
                    accum_out=x[:, bb : bb + 1],
                )
            # next chunk's transposes/L1 between this chunk's ADD groups,
            # so they are ready before DVE needs them and PE never stalls
            if chunk == prepped and chunk + 1 < len(CHUNKS):
                prep_chunk(chunk + 1)
                prepped += 1

        # ---------------- final: out[b, t] = b2 + W2row . cat(x, uemb) ------
        xT_ps = stage.tile([BPC, FC], F32, tag="stage")
        nc.tensor.transpose(xT_ps[:], x[:], ident[:FC, :FC])
        xT = const.tile([BPC, FC], F32)
        nc.vector.tensor_copy(xT[:], xT_ps[:])

        # batched rounds to minimize DVE<->ACT ping-pong in the serial tail
        scr = const.tile([BPC, T * (FC + D)], F32)
        for t in range(T):
            o = t * (FC + D)
            nc.vector.tensor_tensor(
                out=scr[:, o : o + FC],
                in0=w2g[:, o : o + FC],
                in1=xT[:],
                op=Mult,
            )
            nc.vector.tensor_tensor(
                out=scr[:, o + FC : o + FC + D],
                in0=w2g[:, o + FC : o + FC + D],
                in1=ug[:],
                op=Mult,
            )
        acc = const.tile([BPC, T], F32)
        dummy = scrp.tile([BPC, FC + D], F16, tag="fdum")
        for t in range(T):
            o = t * (FC + D)
            nc.scalar.activation(
                dummy[:], scr[:, o : o + FC + D], Ident,
                accum_out=acc[:, t : t + 1],
            )
        out_sb = const.tile([BPC, T], F32)
        nc.vector.tensor_tensor(out=out_sb[:], in0=acc[:], in1=b2g[:], op=Add)
        nc.sync.dma_start(out=out_d[:, :], in_=out_sb[:])

    nc.finalize()
    return nc


def get_program(cfg=None):
    cfg = dict(CFG if cfg is None else cfg)
    key = tuple(sorted(cfg.items()))
    if key not in _PROG_CACHE:
        _PROG_CACHE[key] = _build_program(cfg)
    return _PROG_CACHE[key]


def make_in_maps(inputs):
    """Shard the full-problem inputs into 8 per-core input maps."""
    seq = np.asarray(inputs["seq_var"]).astype(np.int32)
    usr = np.asarray(inputs["user_var"]).astype(np.int32).reshape(B_FULL, 1)
    itm = np.asarray(inputs["item_var"]).astype(np.int32).reshape(B_FULL, T)
    shared = dict(
        item_emb=np.ascontiguousarray(np.asarray(inputs["item_emb"], np.float32)),
        user_emb=np.ascontiguousarray(np.asarray(inputs["user_emb"], np.float32)),
        W2=np.ascontiguousarray(np.asarray(inputs["W2"], np.float32)),
        b2=np.ascontiguousarray(np.asarray(inputs["b2"], np.float32).reshape(-1, 1)),
        W1=np.ascontiguousarray(np.asarray(inputs["W1"], np.float32)),
        b1=np.ascontiguousarray(np.asarray(inputs["b1"], np.float32).reshape(FC, 1)),
        Wf2=np.ascontiguousarray(np.asarray(inputs["Wf2"], np.float32)),
        bf2=np.ascontiguousarray(np.asarray(inputs["bf2"], np.float32).reshape(FC, 1)),
        ident128=np.eye(128, dtype=np.float32),
    )
    in_maps = []
    for c in range(N_CORES):
        rows = slice(c * BPC, (c + 1) * BPC)
        flat = seq[rows].reshape(NROW)               # (b*L + l) order
        seq_pm = np.ascontiguousarray(flat.reshape(NTILE, 128).T)  # [128, 16]
        in_maps.append(
            dict(
                shared,
                seq_idx=seq_pm,
                user_idx=np.ascontiguousarray(usr[rows]),
                item_idx=np.ascontiguousarray(itm[rows]),
            )
        )
    return in_maps


def run_sharded(inputs, cfg=None, trace=False, **kwargs):
    nc = get_program(cfg)
    in_maps = make_in_maps(inputs)
    res = run_bass_kernel_spmd(nc, in_maps, list(range(N_CORES)), trace=trace, **kwargs)
    out = np.concatenate([r["out"] for r in res.results], axis=0)
    return out, res


def kernel(**inputs) -> np.ndarray:
    out, _ = run_sharded(inputs)
    return out


# revision 43
# speedup vs baseline: 1.0360x; 1.0360x over previous
"""CosRec-style pairwise-MLP recommender kernel for 8 Trainium2 NeuronCores.

Reference computation (per batch element b, L=32, D=64, FC=100):
    embs   = item_emb[seq_var]                      [B, L, D]
    A      = embs @ Wa^T + b1 (Wa = W1[:, :D])      [B, L, FC]
    Bm     = embs @ Wb^T  (Wb = W1[:, D:])          [B, L, FC]
    h1     = relu(A[:,None,:,:] + Bm[:,:,None,:])   [B, L, L, FC]
    h2     = relu(h1 @ Wf2^T + bf2)                 [B, L, L, FC]
    x      = h2.sum((1, 2))                         [B, FC]
    out[b,t] = b2[item_var[b,t]] + W2[item_var[b,t]] . cat(x[b], user_emb[user_var[b]])

Strategy: data-parallel over batch (64 examples/core).  Hard-won placement
rules for this runtime/HW:
  - GpSimd must never stream bulk elementwise data (shares an SBUF port
    with the DVE); it only issues the indirect-DMA gathers.  Multi-column
    offset APs are broken in the SWDGE path, so gathers stay one offset
    column per instruction, issued before anything else on that queue
    (the identity matrix comes in as an input so make_identity's iota
    does not block the gathers).
  - DVE accum_out / tensor_tensor_reduce are broken; only ScalarE
    activation accum_out works => relu2(+bf2)+pair-sum lives on ACT.
  - fp16 Wf2 fails the 2e-2 gate (weight rounding error is systematic
    across the 1024-term positive pair-sum); f32r moving operands at
    >=256 cols already run 1 col/cycle, so everything stays f32r.
  - The outer-sum broadcast ADD can never exceed DVE 1x mode (the Bm
    operand has innermost stride 0). Both L1 evictions run on DVE (ACT
    is the relu2 bottleneck); pa gets +b1 fused via a 2D
    scalar_tensor_tensor.
Main loop per ADD group (1-4 examples, small ramp-in/drain-out groups):
one DVE TT ADD -> pre [100, gs*1024] f32r, relu1 in place (DVE 2x
tensor_scalar or ACT, balance knob), then per example 2 f32r matmuls
into PSUM and one ACT relu+bias+accum -> x column.  Chunks are
software-pipelined: chunk c+1's transposes/L1 are emitted between chunk
c's ADD groups so DVE never waits on L1.  Knob values are phase-critical
(+-10us swings): r1_act_num=5, h2_bufs=3/stage_bufs=2, pre_bufs=6.
"""

import os
import sys

import numpy as np

sys.path.insert(0, "/opt/trn_rl_repo")

import concourse.bass as bass
import concourse.tile as tile
from concourse import bacc, mybir
from concourse.bass_utils import run_bass_kernel_spmd
from contextlib import ExitStack

N_CORES = 8
B_FULL = 512
BPC = B_FULL // N_CORES  # 64 examples per core
L = 32
D = 64
FC = 100
T = 3
NROW = BPC * L           # 2048 gathered rows per core
NTILE = NROW // 128      # 16 gather tiles
F32 = mybir.dt.float32
F32R = mybir.dt.float32r
BF16 = mybir.dt.bfloat16
F16 = mybir.dt.float16
I32 = mybir.dt.int32

# ---- tunables -------------------------------------------------------------
CFG = dict(
    r1_act_num=5,      # of 16 4-example groups, how many run relu1 on ACT
    r1_tail_dve=2,     # last N groups always use DVE relu1 (ACT drains relu2)
    bstep=4,           # examples per ADD/relu1 group
    h2_bufs=3,         # PSUM slots of [100, 1024] (2 banks each)
    stage_bufs=2,      # shared PSUM ring for transposes + L1 out (1 bank each)
    pre_bufs=6,
)

_PROG_CACHE = {}


def _build_program(cfg):
    nc = bacc.Bacc()

    seq_idx = nc.dram_tensor("seq_idx", [128, NTILE], I32, kind="ExternalInput")
    user_idx = nc.dram_tensor("user_idx", [BPC, 1], I32, kind="ExternalInput")
    item_idx = nc.dram_tensor("item_idx", [BPC, T], I32, kind="ExternalInput")
    item_emb = nc.dram_tensor("item_emb", [100000, D], F32, kind="ExternalInput")
    user_emb = nc.dram_tensor("user_emb", [100000, D], F32, kind="ExternalInput")
    W2 = nc.dram_tensor("W2", [100000, FC + D], F32, kind="ExternalInput")
    b2 = nc.dram_tensor("b2", [100000, 1], F32, kind="ExternalInput")
    W1 = nc.dram_tensor("W1", [FC, 2 * D], F32, kind="ExternalInput")
    b1 = nc.dram_tensor("b1", [FC, 1], F32, kind="ExternalInput")
    Wf2 = nc.dram_tensor("Wf2", [FC, FC], F32, kind="ExternalInput")
    bf2 = nc.dram_tensor("bf2", [FC, 1], F32, kind="ExternalInput")
    ident_d = nc.dram_tensor("ident128", [128, 128], F32, kind="ExternalInput")
    out_d = nc.dram_tensor("out", [BPC, T], F32, kind="ExternalOutput")

    Relu = mybir.ActivationFunctionType.Relu
    Ident = mybir.ActivationFunctionType.Identity
    Add = mybir.AluOpType.add
    Mult = mybir.AluOpType.mult

    BS = cfg["bstep"]               # examples per ADD group (steady state)
    # chunk layout (examples per L1 chunk): small first chunks so the first
    # ADD starts as soon as the first gathered tile lands
    CHUNKS = [4, 4] + [8] * 7
    assert sum(CHUNKS) == BPC
    # group sizes per chunk: tiny ramp-in (ACT's first relu2 comes ~6us
    # earlier) and a fine drain-out (shorter serial tail after the last ADD)
    GSIZES = [[1, 1, 2], [2, 2]] + [[4] * (c // 4) for c in CHUNKS[2:-1]] + [
        [4, 2, 1, 1]
    ]
    GROUPS = []  # (chunk, b0, size)
    b0 = 0
    for c, sizes in enumerate(GSIZES):
        assert sum(sizes) == CHUNKS[c]
        for s in sizes:
            GROUPS.append((c, b0, s))
            b0 += s
    NGRP = len(GROUPS)
    # relu1 engine per group: k ACT groups spread over all but the last
    # r1_tail_dve groups (ACT must drain relu2 at the end)
    k = cfg["r1_act_num"]
    nfree = NGRP - cfg["r1_tail_dve"]
    r1_act = [
        g < nfree and ((g + 1) * k) // nfree > (g * k) // nfree for g in range(NGRP)
    ]

    with ExitStack() as ctx:
        tc = ctx.enter_context(tile.TileContext(nc))
        const = ctx.enter_context(tc.tile_pool(name="const", bufs=1))
        prep = ctx.enter_context(tc.tile_pool(name="pre", bufs=cfg["pre_bufs"]))
        scrp = ctx.enter_context(tc.tile_pool(name="scr", bufs=2))
        stage = ctx.enter_context(
            tc.tile_pool(name="stage", bufs=cfg["stage_bufs"], space="PSUM")
        )
        ps2 = ctx.enter_context(
            tc.tile_pool(name="ps2", bufs=cfg["h2_bufs"], space="PSUM")
        )

        # ---------------- gathers first: longest GpSimd-serial chain --------
        idx_sb = const.tile([128, NTILE], I32)
        nc.sync.dma_start(out=idx_sb[:], in_=seq_idx[:, :])
        uidx_sb = const.tile([BPC, 1], I32)
        nc.sync.dma_start(out=uidx_sb[:], in_=user_idx[:, :])
        iidx_sb = const.tile([BPC, T], I32)
        nc.sync.dma_start(out=iidx_sb[:], in_=item_idx[:, :])

        # gathers cast fp32->fp16 in the DMA (SWDGE): halves the SBUF write
        # traffic that contends with DVE's 2-port modes (embedding values in
        # fp16 cost ~1e-3 rel err; the gate is 2e-2)
        g_all = const.tile([128, NTILE * D], F16)
        for t in range(NTILE):
            nc.gpsimd.indirect_dma_start(
                out=g_all[:, t * D : (t + 1) * D],
                out_offset=None,
                in_=item_emb[:, :],
                in_offset=bass.IndirectOffsetOnAxis(ap=idx_sb[:, t : t + 1], axis=0),
            )
        # final-stage gathers queue behind; they overlap the main loop
        ug = const.tile([BPC, D], F32)
        nc.gpsimd.indirect_dma_start(
            out=ug[:],
            out_offset=None,
            in_=user_emb[:, :],
            in_offset=bass.IndirectOffsetOnAxis(ap=uidx_sb[:, 0:1], axis=0),
        )
        w2g = const.tile([BPC, T * (FC + D)], F32)
        for t in range(T):
            nc.gpsimd.indirect_dma_start(
                out=w2g[:, t * (FC + D) : (t + 1) * (FC + D)],
                out_offset=None,
                in_=W2[:, :],
                in_offset=bass.IndirectOffsetOnAxis(ap=iidx_sb[:, t : t + 1], axis=0),
            )
        b2g = const.tile([BPC, T], F32)
        for t in range(T):
            nc.gpsimd.indirect_dma_start(
                out=b2g[:, t : t + 1],
                out_offset=None,
                in_=b2[:, :],
                in_offset=bass.IndirectOffsetOnAxis(ap=iidx_sb[:, t : t + 1], axis=0),
            )

        # ---------------- constants & weights ----------------
        ident = const.tile([128, 128], F32)
        nc.sync.dma_start(out=ident[:], in_=ident_d[:, :])
        ident16 = const.tile([128, 128], F16)
        nc.vector.tensor_copy(ident16[:], ident[:])
        w1_sb = const.tile([FC, 2 * D], F32)
        nc.sync.dma_start(out=w1_sb[:], in_=W1[:, :])
        wf2_sb = const.tile([FC, FC], F32)
        nc.sync.dma_start(out=wf2_sb[:], in_=Wf2[:, :])
        b1_sb = const.tile([FC, 1], F32)
        nc.sync.dma_start(out=b1_sb[:], in_=b1[:, :])
        bf2_sb = const.tile([FC, 1], F32)
        nc.sync.dma_start(out=bf2_sb[:], in_=bf2[:, :])

        # WaT/WbT: [64, 100] = (W1[:, :D]).T and (W1[:, D:]).T
        waT = const.tile([D, FC], F32R)
        wbT = const.tile([D, FC], F32R)
        for half, dst in ((0, waT), (1, wbT)):
            w1h_ps = stage.tile([D, FC], F32, tag="stage")
            nc.tensor.transpose(
                w1h_ps[:], w1_sb[:, half * D : (half + 1) * D], ident[:FC, :FC]
            )
            nc.vector.tensor_copy(dst[:], w1h_ps[:])

        # Wf2T: [100, 100] = Wf2.T
        wf2t_ps = stage.tile([FC, FC], F32, tag="stage")
        nc.tensor.transpose(wf2t_ps[:], wf2_sb[:], ident[:FC, :FC])
        wf2t = const.tile([FC, FC], F32R)
        nc.vector.tensor_copy(wf2t[:], wf2t_ps[:])

        embsT = const.tile([D, NROW], F32R)
        A_sb = const.tile([FC, NROW], F32)   # A' = embs@Wa^T + b1 (bias folded)
        Bm_sb = const.tile([FC, NROW], F32)  # Bm = embs@Wb^T
        x = const.tile([FC, BPC], F32)       # x[:, b] = sum_{a,c} h2[b, a, c, :]
        zeros = const.tile([FC, 1], F32)
        nc.vector.memset(zeros[:], 0.0)

        chunk_b0 = [sum(CHUNKS[:i]) for i in range(len(CHUNKS))]  # first example

        def prep_chunk(chunk):
            """Transpose + layer 1 + DVE evictions for one chunk."""
            cb = CHUNKS[chunk]
            cw = cb * L
            r0 = chunk_b0[chunk] * L          # first gathered row
            tp = stage.tile([D, 256], F16, tag="stage", name=f"tp{chunk}")
            for i in range(cw // 128):
                t0 = r0 // 128 + i
                nc.tensor.transpose(
                    tp[:, i * 128 : (i + 1) * 128],
                    g_all[:, t0 * D : (t0 + 1) * D],
                    ident16[:, :],
                )
            sl = slice(r0, r0 + cw)
            nc.vector.tensor_copy(embsT[:, sl], tp[:, 0:cw])
            l1 = stage.tile([FC, 512], F32, tag="stage", name=f"l1_{chunk}")
            nc.tensor.matmul(
                l1[:, 0:cw], lhsT=waT[:], rhs=embsT[:, sl], start=True, stop=True
            )
            nc.tensor.matmul(
                l1[:, cw : 2 * cw], lhsT=wbT[:], rhs=embsT[:, sl], start=True, stop=True
            )
            nc.vector.scalar_tensor_tensor(
                out=A_sb[:, sl],
                in0=l1[:, 0:cw],
                scalar=b1_sb[:, 0:1],
                in1=zeros[:, 0:1].to_broadcast([FC, cw]),
                op0=Add,
                op1=Add,
            )
            nc.vector.tensor_copy(Bm_sb[:, sl], l1[:, cw : 2 * cw])

        prep_chunk(0)
        prepped = 0
        for grp, (chunk, b, gs) in enumerate(GROUPS):
            pre = prep.tile([FC, gs * L * L], F32R, tag="pre")
            in0 = (
                A_sb[:, b * L : (b + gs) * L]
                .rearrange("p (j c) -> p j c", j=gs)
                .unsqueeze(2)
                .to_broadcast([FC, gs, L, L])
            )
            in1 = (
                Bm_sb[:, b * L : (b + gs) * L]
                .rearrange("p (j a) -> p j a", j=gs)
                .unsqueeze(3)
                .to_broadcast([FC, gs, L, L])
            )
            nc.vector.tensor_tensor(
                out=pre[:].rearrange("p (j a c) -> p j a c", j=gs, a=L),
                in0=in0,
                in1=in1,
                op=Add,
            )
            # relu1 in place, split fine-grained so layer-2 matmuls of
            # example j start right after relu1(j), not the whole group
            if r1_act[grp]:
                nh = max(1, gs // 2)
                half = gs * L * L // nh
                for h in range(nh):
                    nc.scalar.activation(
                        pre[:, h * half : (h + 1) * half],
                        pre[:, h * half : (h + 1) * half],
                        Relu,
                    )
            else:
                nj = 2 if gs == 4 else 1
                step = gs * L * L // nj
                for j in range(nj):
                    nc.vector.tensor_scalar_max(
                        pre[:, j * step : (j + 1) * step],
                        pre[:, j * step : (j + 1) * step],
                        0.0,
                    )
            for j in range(gs):
                bb = b + j
                h2p = ps2.tile([FC, L * L], F32, tag="ps2")
                for h in range(2):
                    nc.tensor.matmul(
                        h2p[:, h * 512 : (h + 1) * 512],
                        lhsT=wf2t[:],
                        rhs=pre[:, j * L * L + h * 512 : j * L * L + (h + 1) * 512],
                        start=True,
                        stop=True,
                    )
                # relu2(+bf2) with fused pair-sum accumulation -> x[:, bb]
                h2s = scrp.tile([FC, L * L], F16, tag="h2s")
                nc.scalar.activation(
                    h2s[:], h2p[:], Relu,
                    bias=bf2_sb[:, 0:1],
                    accum_out=x[:, bb : bb + 1],
                )
            # next chunk's transposes/L1 between this chunk's ADD groups,
            # so they are ready before DVE needs them and PE never stalls
            if chunk == prepped and chunk + 1 < len(CHUNKS):
                prep_chunk(chunk + 1)
                prepped += 1

        # ---------------- final: out[b, t] = b2 + W2row . cat(x, uemb) ------
        xT_ps = stage.tile([BPC, FC], F32, tag="stage")
        nc.tensor.transpose(xT_ps[:], x[:], ident[:FC, :FC])
        xT = const.tile([BPC, FC], F32)
        nc.vector.tensor_copy(xT[:], xT_ps[:])

        # batched rounds to minimize DVE<->ACT ping-pong in the serial tail
        scr = const.tile([BPC, T * (FC + D)], F32)
        for t in range(T):
            o = t * (FC + D)
            nc.vector.tensor_tensor(
                out=scr[:, o : o + FC],
                in0=w2g[:, o : o + FC],
                in1=xT[:],
                op=Mult,
            )
            nc.vector.tensor_tensor(
                out=scr[:, o + FC : o + FC + D],
                in0=w2g[:, o + FC : o + FC + D],
                in1=ug[:],
                op=Mult,
            )
        acc = const.tile([BPC, T], F32)
        dummy = scrp.tile([BPC, FC + D], F16, tag="fdum")
        for t in range(T):
            o = t * (FC + D)
            nc.scalar.activation(
                dummy[:], scr[:, o : o + FC + D], Ident,
                accum_out=acc[:, t : t + 1],
            )
        out_sb = const.tile([BPC, T], F32)
        nc.vector.tensor_tensor(out=out_sb[:], in0=acc[:], in1=b2g[:], op=Add)
        nc.sync.dma_start(out=out_d[:, :], in_=out_sb[:])

    nc.finalize()
    return nc


def get_program(cfg=None):
    cfg = dict(CFG if cfg is None else cfg)
    key = tuple(sorted(cfg.items()))
    if key not in _PROG_CACHE:
        _PROG_CACHE[key] = _build_program(cfg)
    return _PROG_CACHE[key]


def make_in_maps(inputs):
    """Shard the full-problem inputs into 8 per-core input maps."""
    seq = np.asarray(inputs["seq_var"]).astype(np.int32)
    usr = np.asarray(inputs["user_var"]).astype(np.int32).reshape(B_FULL, 1)
    itm = np.asarray(inputs["item_var"]).astype(np.int32).reshape(B_FULL, T)
    shared = dict(
        item_emb=np.ascontiguousarray(np.asarray(inputs["item_emb"], np.float32)),
        user_emb=np.ascontiguousarray(np.asarray(inputs["user_emb"], np.float32)),
        W2=np.ascontiguousarray(np.asarray(inputs["W2"], np.float32)),
        b2=np.ascontiguousarray(np.asarray(inputs["b2"], np.float32).reshape(-1, 1)),
        W1=np.ascontiguousarray(np.asarray(inputs["W1"], np.float32)),
        b1=np.ascontiguousarray(np.asarray(inputs["b1"], np.float32).reshape(FC, 1)),
        Wf2=np.ascontiguousarray(np.asarray(inputs["Wf2"], np.float32)),
        bf2=np.ascontiguousarray(np.asarray(inputs["bf2"], np.float32).reshape(FC, 1)),
        ident128=np.eye(128, dtype=np.float32),
    )
    in_maps = []
    for c in range(N_CORES):
        rows = slice(c * BPC, (c + 1) * BPC)
        flat = seq[rows].reshape(NROW)               # (b*L + l) order
        seq_pm = np.ascontiguousarray(flat.reshape(NTILE, 128).T)  # [128, 16]
        in_maps.append(
            dict(
                shared,
                seq_idx=seq_pm,
                user_idx=np.ascontiguousarray(usr[rows]),
                item_idx=np.ascontiguousarray(itm[rows]),
            )
        )
    return in_maps


def run_sharded(inputs, cfg=None, trace=False, **kwargs):
    nc = get_program(cfg)
    in_maps = make_in_maps(inputs)
    res = run_bass_kernel_spmd(nc, in_maps, list(range(N_CORES)), trace=trace, **kwargs)
    out = np.concatenate([r["out"] for r in res.results], axis=0)
    return out, res


def kernel(**inputs) -> np.ndarray:
    out, _ = run_sharded(inputs)
    return out
